# revision 1
# baseline (speedup 1.0000x reference)
"""Multi-head attention Trainium2 kernel (8 NeuronCores, data-parallel over batch).

Per-core program (2 batches per core):
  x [2048, 512] (row-major [t, c] per batch)
  -> PE-transpose to xT [c, t] (f32)
  -> QKV projections in float32r (FP22, full-rate): qT/kT [d, t] per head-pair,
     V [t, hd] (stored bf16)
  -> scores S^T [s, tq] per (pair, s-tile, head), K=64 row-tiled so the two
     heads of a pair run concurrently in the PE array (f32r)
  -> exp on ScalarE (scale=1/8 folded in), PSUM -> SBUF bf16
  -> PV + rowsum in bf16, column-tiled: O pair occupies PSUM partitions
     0:64 / 64:128, rowsum broadcast comes from an all-ones stationary
  -> normalize on VectorE (reciprocal + multiply) -> O^T [hd, t] f32r
  -> output projection f32r + bias add -> y [2048, 512]
"""
import sys
import os

sys.path.insert(0, "/opt/trn_rl_repo")
import numpy as np

B, C, HH, WW = 16, 512, 32, 32
T = HH * WW              # 1024
NH, HD = 8, 64
BL = 2                   # batches per core
NCORES = 8

_CACHE = {}


def _build_nc():
    import concourse.bacc as bacc
    import concourse.mybir as mybir
    import concourse.tile as tile
    from concourse import masks

    f32 = mybir.dt.float32
    f32r = mybir.dt.float32r
    bf16 = mybir.dt.bfloat16
    Exp = mybir.ActivationFunctionType.Exp

    nc = bacc.Bacc("TRN2", target_bir_lowering=False, debug=False, num_devices=NCORES)
    x = nc.dram_tensor("x", [BL * T, C], f32, kind="ExternalInput").ap()
    wq = nc.dram_tensor("wq", [128, 2048], f32, kind="ExternalInput").ap()
    wk = nc.dram_tensor("wk", [128, 2048], f32, kind="ExternalInput").ap()
    wv = nc.dram_tensor("wv", [128, 2048], f32, kind="ExternalInput").ap()
    wp = nc.dram_tensor("wp", [128, 2048], f32, kind="ExternalInput").ap()
    bp = nc.dram_tensor("bp", [1, C], f32, kind="ExternalInput").ap()
    y = nc.dram_tensor("y", [BL * T, C], f32, kind="ExternalOutput").ap()

    with tile.TileContext(nc) as tc:
        with tc.tile_pool(name="const", bufs=1) as cpool, \
             tc.tile_pool(name="xnat", bufs=3) as xn_pool, \
             tc.tile_pool(name="xt", bufs=1) as xt_pool, \
             tc.tile_pool(name="qk", bufs=8) as qk_pool, \
             tc.tile_pool(name="vv", bufs=16) as v_pool, \
             tc.tile_pool(name="pp", bufs=28) as p_pool, \
             tc.tile_pool(name="ot", bufs=5) as ot_pool, \
             tc.tile_pool(name="rc", bufs=2) as rc_pool, \
             tc.tile_pool(name="yy", bufs=3) as y_pool, \
             tc.tile_pool(name="ps", bufs=4, space="PSUM") as ps_pool:

            wq_s = cpool.tile([128, 2048], f32r, tag="wq")
            wk_s = cpool.tile([128, 2048], f32r, tag="wk")
            wv_s = cpool.tile([128, 2048], f32r, tag="wv")
            wp_s = cpool.tile([128, 2048], f32r, tag="wp")
            nc.sync.dma_start(wq_s[:], wq.bitcast(f32r))
            nc.sync.dma_start(wk_s[:], wk.bitcast(f32r))
            nc.sync.dma_start(wv_s[:], wv.bitcast(f32r))
            nc.sync.dma_start(wp_s[:], wp.bitcast(f32r))
            bias_b = cpool.tile([128, C], f32, tag="bias")
            nc.sync.dma_start(bias_b[:], bp.to_broadcast([128, C]))
            ones_bf = cpool.tile([128, HD], bf16, tag="ones")
            nc.gpsimd.memset(ones_bf[:], 1.0)
            ident = cpool.tile([128, 128], f32, tag="ident")
            masks.make_identity(nc, ident[:])

            def prep(b):
                # ---- load + transpose x -> xT [c_local, cc, t] ----
                xts = xt_pool.tile([128, 4, T], f32r, tag="xt", name=f"xts_{b}")
                for tt in range(8):
                    xn = xn_pool.tile([128, C], f32, tag="xn", name=f"xn_{b}_{tt}")
                    nc.sync.dma_start(xn[:], x[b * T + tt * 128: b * T + tt * 128 + 128, :])
                    tr = ps_pool.tile([128, C], f32, tag="ps", name=f"tr_{b}_{tt}")
                    for cc in range(4):
                        nc.tensor.transpose(tr[:, cc * 128:(cc + 1) * 128],
                                            xn[:, cc * 128:(cc + 1) * 128], ident[:])
                    nc.vector.tensor_copy(xts[:, :, tt * 128:(tt + 1) * 128],
                                          tr[:].rearrange("p (cc m) -> p cc m", cc=4))

                # ---- QKV projections ----
                qts, kts = [], []
                for p in range(4):
                    for wi, (wsb, lst) in enumerate(((wq_s, qts), (wk_s, kts))):
                        ps_t = ps_pool.tile([128, T], f32, tag="ps", name=f"qk_{b}_{p}_{wi}")
                        for ch in range(2):
                            for cc in range(4):
                                nc.tensor.matmul(
                                    ps_t[:, ch * 512:(ch + 1) * 512],
                                    wsb[:, cc * 512 + p * 128: cc * 512 + p * 128 + 128],
                                    xts[:, cc, ch * 512:(ch + 1) * 512],
                                    start=(cc == 0), stop=(cc == 3))
                        sb_t = qk_pool.tile([128, T], f32r, tag="qk", name=f"qks_{b}_{p}_{wi}")
                        nc.vector.tensor_copy(sb_t[:], ps_t[:])
                        lst.append(sb_t)
                vts = []
                for st in range(8):
                    ps_t = ps_pool.tile([128, C], f32, tag="ps", name=f"v_{b}_{st}")
                    for cc in range(4):
                        nc.tensor.matmul(ps_t[:],
                                         xts[:, cc, st * 128:(st + 1) * 128],
                                         wv_s[:, cc * 512:(cc + 1) * 512],
                                         start=(cc == 0), stop=(cc == 3))
                    v_t = v_pool.tile([128, C], bf16, tag="v", name=f"vs_{b}_{st}")
                    nc.vector.tensor_copy(v_t[:], ps_t[:])
                    vts.append(v_t)
                return qts, kts, vts

            def attention(b, qts, kts, vts):
                # ---- attention, one head-pair at a time ----
                # Phase 1 per pair: all scores + exp (P~ for the whole pair
                # lives in SBUF).  Phase 2: PV+rowsum in two tq halves so
                # o/r only pin one PSUM bank each, leaving slots for the
                # next pair's scores/exp (and next batch's QKV) to overlap.
                ots = []
                for p in range(4):
                    pjs = {}
                    for j in range(8):
                        s_list = [ps_pool.tile([128, T], f32, tag="ps", name=f"s_{b}_{p}_{j}_{h}")
                                  for h in range(2)]
                        for ch in range(2):
                            for h in range(2):
                                nc.tensor.matmul(
                                    s_list[h][:, ch * 512:(ch + 1) * 512],
                                    kts[p][h * 64:h * 64 + 64, j * 128:(j + 1) * 128],
                                    qts[p][h * 64:h * 64 + 64, ch * 512:(ch + 1) * 512])
                        for h in range(2):
                            p_sb = p_pool.tile([128, T], bf16, tag="p", name=f"p_{b}_{p}_{j}_{h}")
                            nc.scalar.activation(p_sb[:], s_list[h][:], Exp, scale=0.125)
                            pjs[(j, h)] = p_sb
                    ot = ot_pool.tile([128, T], f32r, tag="ot", name=f"ot_{b}_{p}")
                    for tq in range(2):
                        # O pair in bank 0 (cols 0:512), rowsum pair in bank 1
                        # (cols 512:1024): one PSUM slot per tq half, so the
                        # next half's matmuls need not wait for this half's
                        # DVE normalize to release two slots.
                        or_ps = ps_pool.tile([128, 1024], f32, tag="ps", name=f"or_{b}_{p}_{tq}")
                        for j in range(8):
                            for h in range(2):
                                nc.tensor.matmul(
                                    or_ps[h * 64:h * 64 + 64, 0:512],
                                    vts[j][:, (2 * p + h) * 64:(2 * p + h) * 64 + 64],
                                    pjs[(j, h)][:, tq * 512:(tq + 1) * 512],
                                    start=(j == 0), stop=(j == 7),
                                    skip_group_check=True)
                            for h in range(2):
                                nc.tensor.matmul(
                                    or_ps[h * 64:h * 64 + 64, 512:1024],
                                    ones_bf[:, 0:HD],
                                    pjs[(j, h)][:, tq * 512:(tq + 1) * 512],
                                    start=(j == 0), stop=(j == 7),
                                    skip_group_check=True)
                        rec = rc_pool.tile([128, 512], f32, tag="rc", name=f"rec_{b}_{p}_{tq}")
                        nc.vector.reciprocal(rec[:], or_ps[:, 512:1024])
                        nc.vector.tensor_mul(ot[:, tq * 512:(tq + 1) * 512], or_ps[:, 0:512], rec[:])
                    ots.append(ot)
                return ots

            def proj(b, ots):
                # ---- output projection + bias ----
                for tt in range(8):
                    y_ps = ps_pool.tile([128, C], f32, tag="ps", name=f"y_{b}_{tt}")
                    for p in range(4):
                        nc.tensor.matmul(y_ps[:],
                                         ots[p][:, tt * 128:(tt + 1) * 128],
                                         wp_s[:, p * 512:(p + 1) * 512],
                                         start=(p == 0), stop=(p == 3))
                    y_sb = y_pool.tile([128, C], f32, tag="y", name=f"ys_{b}_{tt}")
                    nc.vector.tensor_add(y_sb[:], y_ps[:], bias_b[:])
                    nc.sync.dma_start(y[b * T + tt * 128: b * T + tt * 128 + 128, :], y_sb[:])

            # Emission order: hoist batch 1's load/transpose/QKV before
            # batch 0's projection so the scheduler can fill batch 0's
            # exp-gated attention windows with batch 1 PE work.
            q0 = prep(0)
            ot0 = attention(0, *q0)
            q1 = prep(1)
            proj(0, ot0)
            ot1 = attention(1, *q1)
            proj(1, ot1)

    nc.compile()
    return nc


def _pack_qk(w):
    # [NH, C, HD] -> [c, h*HD+d] -> tiled [c_local, cc, p, m] -> [128, 2048]
    wn = np.transpose(w, (1, 0, 2)).reshape(C, C)
    return np.ascontiguousarray(
        wn.reshape(4, 128, 4, 128).transpose(1, 0, 2, 3).reshape(128, 2048))


def _pack_cn(wn):
    # [C, N] natural -> tiled [c_local, cc, n] -> [128, 2048]
    return np.ascontiguousarray(wn.reshape(4, 128, C).transpose(1, 0, 2).reshape(128, 2048))


def get_nc():
    if "nc" not in _CACHE:
        _CACHE["nc"] = _build_nc()
    return _CACHE["nc"]


def make_in_maps(x, Wq, Wk, Wv, Wproj, bproj):
    x = np.asarray(x, dtype=np.float32)
    wq_t = _pack_qk(np.asarray(Wq, np.float32))
    wk_t = _pack_qk(np.asarray(Wk, np.float32))
    wv_t = _pack_cn(np.transpose(np.asarray(Wv, np.float32), (1, 0, 2)).reshape(C, C))
    wp_t = _pack_cn(np.asarray(Wproj, np.float32))
    bp_t = np.asarray(bproj, np.float32).reshape(1, C)
    in_maps = []
    for i in range(NCORES):
        in_maps.append({
            "x": np.ascontiguousarray(x[BL * i: BL * (i + 1)].reshape(BL * T, C)),
            "wq": wq_t, "wk": wk_t, "wv": wv_t, "wp": wp_t, "bp": bp_t,
        })
    return in_maps


def kernel(x, Wq, Wk, Wv, Wproj, bproj):
    from concourse.bass_utils import run_bass_kernel_spmd

    nc = get_nc()
    in_maps = make_in_maps(x, Wq, Wk, Wv, Wproj, bproj)
    trace = bool(int(os.environ.get("KERNEL_TRACE", "0")))
    res = run_bass_kernel_spmd(nc, in_maps, list(range(NCORES)), trace=trace)
    _CACHE["last_result"] = res
    out = np.empty((B, C, HH, WW), np.float32)
    for i in range(NCORES):
        out[BL * i: BL * (i + 1)] = res.results[i]["y"].reshape(BL, C, HH, WW)
    return out

